# revision 8
# baseline (speedup 1.0000x reference)
"""CaMoE system kernel v2 for 8 Trainium2 NeuronCores.

Token-parallel (256 tok/core) timemix/LN/routing; H-sharded dense experts
with AllGather dispatch + ReduceScatter combine; vocab-split head.

Precision plan (validated in numpy emulation):
- routing-critical GEMMs (timemix both layers, layer-0 experts + bridge):
  bf16x2 3-pass (hi@Whi + lo@Whi + hi@Wlo), error ~1e-6 -> no top-2 flips
- layer-1 experts/bridge: bf16 single pass (post-routing, output-only)
- LN stats + router + gate broadcast: plain fp32 matmuls (exact, small)
- head: bf16
"""
import os
import sys
sys.path.insert(0, "/opt/trn_rl_repo")

import numpy as np
import ml_dtypes

import concourse.bass as bass
import concourse.bacc as bacc
import concourse.mybir as mybir
import concourse.tile as tile
from concourse import bass_utils
from concourse.masks import make_identity

F32 = mybir.dt.float32
BF16 = mybir.dt.bfloat16
AF = mybir.ActivationFunctionType
OP = mybir.AluOpType
AX = mybir.AxisListType

B, T, C, V, L = 2, 1024, 768, 50257, 2
E_R, E_T, E = 6, 2, 8
H, H2 = 4 * C, 2 * C
EPS = 1e-5
NC_ = 8
P = 128
CT = C // P            # 6 channel k-tiles
TOK = (B * T) // NC_   # 256 tokens per core
M = B * T              # 2048
CH = 512               # expert/head chunk
NCH = M // CH          # 4
RPC = CH // TOK        # ranks per chunk = 2
HS = H // NC_          # 384
H2S = H2 // NC_        # 192
RWKV_MT = E_R * HS // P   # 18
TR_MT = E_T * H2S // P    # 3
W2_KT = RWKV_MT + TR_MT   # 21
VS = 6283
VSP = 6400
HEAD_MT = VSP // P        # 50

_CACHE = {}


def _build():
    nc = bacc.Bacc("TRN2", target_bir_lowering=False, debug=False, num_devices=NC_)

    def din(name, shape, dt=F32):
        return nc.dram_tensor(name, list(shape), dt, kind="ExternalInput")

    x0T = din("x0T", [CT, P, TOK])
    wmix = din("wmix", [L, 4, CT, P, 2, CT, P], BF16)  # [l,proj,m,p,hh,kt,n]
    dsig = din("dsig", [L, P, CT])
    scanP = din("scanP", [L, P, CT, NC_])
    ln1s = din("ln1s", [L, P, CT]); ln1b = din("ln1b", [L, P, CT])
    ln2s = din("ln2s", [L, P, CT]); ln2b = din("ln2b", [L, P, CT])
    lnos = din("lnos", [P, CT]); lnob = din("lnob", [P, CT])
    routerW = din("routerW", [L, C, 16])
    confb = din("confb", [L, P, E])
    sharesb = din("sharesb", [L, P, E])
    bridgeW = din("bridgeW", [CT, P, 2, 2 * CT, P], BF16)
    w1s = din("w1s", [L, RWKV_MT, P, 2, CT, P], BF16)
    w2s = din("w2s", [L, RWKV_MT, P, 2, CT, P], BF16)
    aps_ = din("aps", [L, 2 * TR_MT, P, 2, CT, P], BF16)
    bs_ = din("bs", [L, TR_MT, P, 2, CT, P], BF16)
    headw = din("headw", [HEAD_MT, P, CT, P], BF16)
    sel8c = din("sel8c", [E, E, P])
    onescol = din("onescol", [P, 1])
    onesrow = din("onesrow", [1, P])

    logitsT = nc.dram_tensor("logitsT", [VSP, M], BF16, kind="ExternalOutput")

    fin_in = nc.dram_tensor("fin_in", [P, CT], F32)
    fin_out = nc.dram_tensor("fin_out", [NC_ * P, CT], F32, addr_space="Shared")
    bids_in = nc.dram_tensor("bids_in", [TOK, E], F32)
    bids_out = nc.dram_tensor("bids_out", [M, E], F32, addr_space="Shared")
    h_in = nc.dram_tensor("h_in", [CT, P, TOK], F32)
    h_out = nc.dram_tensor("h_out", [NC_, CT, P, TOK], F32, addr_space="Shared")
    pf_in = nc.dram_tensor("pf_in", [CT, P, TOK], F32)
    pf_out = nc.dram_tensor("pf_out", [NC_, CT, P, TOK], F32,
                            addr_space="Shared")
    rs_in = nc.dram_tensor("rs_in", [NC_, CT, P, TOK], F32)
    rs_out = nc.dram_tensor("rs_out", [CT, P, TOK], F32)
    xf_in = nc.dram_tensor("xf_in", [CT, P, TOK], F32)
    xf_out = nc.dram_tensor("xf_out", [NC_, CT, P, TOK], F32, addr_space="Shared")

    RG = [list(range(NC_))]
    ctxs = []

    tc = tile.TileContext(nc)
    tc.__enter__()
    try:
        def pool(name, bufs, space="SBUF"):
            p_ = tc.tile_pool(name=name, bufs=bufs, space=space)
            ctxs.append(p_)
            return p_.__enter__()

        cpool = pool("const", 1)
        xpool = pool("xp", 1)
        wkp = pool("wk", 1)
        exp = pool("ex", 1)
        slabp = pool("slab", 3)
        wst = pool("wst", 2)
        stg = pool("stg", 2)
        psp = pool("ps", 2, space="PSUM")
        w2p = pool("w2p", 1, space="PSUM")

        ident = cpool.tile([P, P], F32)
        make_identity(nc, ident[:])
        ones_col = cpool.tile([P, 1], F32)
        nc.sync.dma_start(out=ones_col[:], in_=onescol.ap())
        ones_row = cpool.tile([1, P], F32)
        nc.sync.dma_start(out=ones_row[:], in_=onesrow.ap())
        sel8b = cpool.tile([E, E, P], BF16)
        nc.gpsimd.dma_start(out=sel8b[:], in_=sel8c.ap())

        x = xpool.tile([P, CT, TOK], F32, tag="x")
        nc.sync.dma_start(out=x[:], in_=x0T.ap().rearrange("kt p t -> p kt t"))
        vf = xpool.tile([P, CT, TOK], F32, tag="vf")

        def aux_ps():
            return psp.tile([P, 512], F32, space="PSUM", tag="mm1", name="aux")

        def split256(src, tag):
            """src [P, CT, TOK] f32 -> pair [P, 2, CT, TOK] bf16."""
            pair = wkp.tile([P, 2, CT, TOK], BF16, tag=tag)
            nc.vector.tensor_copy(pair[:, 0], src[:])
            nc.vector.tensor_tensor(out=pair[:, 1], in0=src[:], in1=pair[:, 0],
                                    op=OP.subtract)
            return pair

        def layer_norm(src, s_ap, b_ap):
            """src [P, CT, TOK] f32 -> normalized f32 tile (tag lnout)."""
            x2 = wkp.tile([P, CT, TOK], F32, tag="lnx2", name="x2")
            nc.vector.tensor_tensor(out=x2[:], in0=src[:], in1=src[:], op=OP.mult)
            aux = aux_ps()
            ps_st = (aux[0:1, 0:TOK], aux[0:1, TOK:2 * TOK])
            for k in range(CT):
                nc.tensor.matmul(ps_st[0], ones_col[:], src[:, k, :],
                                 start=(k == 0), stop=(k == CT - 1))
            for k in range(CT):
                nc.tensor.matmul(ps_st[1], ones_col[:], x2[:, k, :],
                                 start=(k == 0), stop=(k == CT - 1))
            stats = wkp.tile([1, 2, TOK], F32, tag="lnsts")
            mean, ex2 = (stats[:, i, :] for i in range(2))
            nc.vector.tensor_scalar(out=mean, in0=ps_st[0], scalar1=1.0 / C,
                                    scalar2=None, op0=OP.mult)
            nc.vector.tensor_scalar(out=ex2, in0=ps_st[1], scalar1=1.0 / C,
                                    scalar2=None, op0=OP.mult)
            nr = wkp.tile([1, 2, TOK], F32, tag="lnnr")
            nc.vector.tensor_tensor(out=nr[:, 0, :], in0=mean, in1=mean,
                                    op=OP.mult)
            nc.vector.tensor_tensor(out=ex2, in0=ex2, in1=nr[:, 0, :],
                                    op=OP.subtract)
            nc.vector.tensor_scalar(out=ex2, in0=ex2, scalar1=EPS, scalar2=None,
                                    op0=OP.add)
            nc.scalar.activation(nr[:, 0, :], ex2, AF.Sqrt)
            nc.vector.reciprocal(nr[:, 0, :], nr[:, 0, :])
            nc.vector.tensor_scalar(out=nr[:, 1, :], in0=mean, scalar1=-1.0,
                                    scalar2=None, op0=OP.mult)
            aux2 = aux_ps()
            ps_b = (aux2[:, 0:TOK], aux2[:, TOK:2 * TOK])
            nc.tensor.matmul(ps_b[0], ones_row[:], nr[:, 0, :],
                             start=True, stop=True)
            nc.tensor.matmul(ps_b[1], ones_row[:], nr[:, 1, :],
                             start=True, stop=True)
            out = wkp.tile([P, CT, TOK], F32, tag="lnout")
            for k in range(CT):
                nc.vector.tensor_tensor(out=out[:, k, :], in0=src[:, k, :],
                                        in1=ps_b[1], op=OP.add)
                nc.vector.tensor_tensor(out=out[:, k, :], in0=out[:, k, :],
                                        in1=ps_b[0], op=OP.mult)
            st = wkp.tile([P, 2, CT], F32, tag="lnsc")
            nc.sync.dma_start(out=st[:, 0, :], in_=s_ap)
            nc.sync.dma_start(out=st[:, 1, :], in_=b_ap)
            for k in range(CT):
                nc.vector.tensor_scalar(out=out[:, k, :], in0=out[:, k, :],
                                        scalar1=st[:, 0, k:k + 1],
                                        scalar2=st[:, 1, k:k + 1],
                                        op0=OP.mult, op1=OP.add)
            return out

        # 3-pass bf16x2 projection: out[P, CT, TOK] from pair rhs
        def proj3(w_ap2, rhs_pair, out, act=None, accum_into=None):
            """w_ap2: dram [2, C, C]-like; rhs_pair [P, 2, CT, TOK] bf16."""
            for m in range(CT):
                w_t = wst.tile([P, 2, CT, P], BF16, tag="wpair", bufs=3)
                nc.sync.dma_start(out=w_t[:], in_=w_ap2[m])
                ps = w2p.tile([P, CH], F32, space="PSUM", tag=f"w2_{m}")
                i = 0
                for (wh, xh) in ((0, 0), (0, 1), (1, 0)):
                    for k in range(CT):
                        nc.tensor.matmul(ps[:, 0:TOK], w_t[:, wh, k, :],
                                         rhs_pair[:, xh, k, :],
                                         start=(i == 0), stop=(i == 3 * CT - 1))
                        i += 1
                if accum_into is not None:
                    nc.vector.tensor_tensor(out=accum_into[:, m, :],
                                            in0=accum_into[:, m, :],
                                            in1=ps[:, 0:TOK], op=OP.add)
                elif act is not None:
                    nc.scalar.activation(out[:, m, :], ps[:, 0:TOK], act)
                else:
                    nc.vector.tensor_copy(out[:, m, :], ps[:, 0:TOK])

        for l in range(L):
            xln = layer_norm(x, ln1s.ap()[l], ln1b.ap()[l])
            xlnp = split256(xln, "pairA")

            sigr = wkp.tile([P, CT, TOK], F32, tag="sigr")
            kk = wkp.tile([P, CT, TOK], F32, tag="kk")
            vv = wkp.tile([P, CT, TOK], F32, tag="vv")
            proj3(wmix.ap()[l, 1], xlnp, kk)
            proj3(wmix.ap()[l, 2], xlnp, vv)
            if l == 0:
                nc.vector.tensor_copy(vf[:], vv[:])
            else:
                nc.vector.tensor_tensor(out=vv[:], in0=vv[:], in1=vf[:], op=OP.add)
                nc.vector.tensor_scalar(out=vv[:], in0=vv[:], scalar1=0.5,
                                        scalar2=None, op0=OP.mult)
            kv = kk
            nc.vector.tensor_tensor(out=kv[:], in0=kk[:], in1=vv[:], op=OP.mult)

            dtile = wkp.tile([P, CT], F32, tag="dt")
            nc.sync.dma_start(out=dtile[:], in_=dsig.ap()[l])
            states = wkp.tile([P, CT, TOK], F32, tag="states")
            for k in range(CT):
                nc.vector.tensor_tensor_scan(
                    states[:, k, :], dtile[:, k:k + 1].to_broadcast([P, TOK]),
                    kv[:, k, :], 0.0, op0=OP.mult, op1=OP.add)
            fin = wkp.tile([P, CT], F32, tag="fin")
            for k in range(CT):
                nc.vector.tensor_copy(fin[:, k:k + 1], states[:, k, TOK - 1:TOK])
            nc.sync.dma_start(out=fin_in.ap(), in_=fin[:])
            nc.gpsimd.collective_compute(
                "AllGather", OP.bypass, replica_groups=RG,
                ins=[fin_in.ap().opt()], outs=[fin_out.ap().opt()])
            proj3(wmix.ap()[l, 0], xlnp, sigr, act=AF.Sigmoid)
            lt = wkp.tile([P, CT, NC_], F32, tag="lfin")
            nc.sync.dma_start(out=lt[:],
                              in_=fin_out.ap().rearrange("(m p) kt -> p kt m", p=P))
            pt = wkp.tile([P, CT, NC_], F32, tag="pfin")
            nc.sync.dma_start(out=pt[:], in_=scanP.ap()[l])
            nc.vector.tensor_tensor(out=lt[:], in0=lt[:], in1=pt[:], op=OP.mult)
            init = wkp.tile([P, CT], F32, tag="init")
            nc.vector.tensor_reduce(init[:], lt[:], axis=AX.X, op=OP.add)
            for k in range(CT):
                nc.vector.tensor_tensor_scan(
                    states[:, k, :], dtile[:, k:k + 1].to_broadcast([P, TOK]),
                    kv[:, k, :], init[:, k:k + 1], op0=OP.mult, op1=OP.add)
            satt = wkp.tile([P, CT, TOK], F32, tag="vv", name="satt")
            nc.vector.tensor_tensor(out=satt[:], in0=sigr[:], in1=states[:],
                                    op=OP.mult)
            sattp = split256(satt, "pairB")
            proj3(wmix.ap()[l, 3], sattp, None, accum_into=x)

            h = layer_norm(x, ln2s.ap()[l], ln2b.ap()[l])
            nc.sync.dma_start(out=h_in.ap().rearrange("kt p t -> p kt t"),
                              in_=h[:])
            nc.gpsimd.collective_compute(
                "AllGather", OP.bypass, replica_groups=RG,
                ins=[h_in.ap().opt()], outs=[h_out.ap().opt()])

            # router (fp32, exact)
            rwt = wkp.tile([P, CT, 16], F32, tag="rwt")
            nc.sync.dma_start(out=rwt[:],
                              in_=routerW.ap()[l].rearrange("(kt p) n -> p kt n", p=P))
            se_t = wkp.tile([P, 2, E], F32, tag="sht")
            nc.sync.dma_start(out=se_t[:, 0, :], in_=sharesb.ap()[l])
            nc.sync.dma_start(out=se_t[:, 1, :], in_=confb.ap()[l])
            bids_sb = wkp.tile([P, TOK // P, E], F32, tag="bids")
            for m in range(TOK // P):
                aux = aux_ps()
                ps = aux[:, 0:16]
                for k in range(CT):
                    nc.tensor.matmul(ps, h[:, k, m * P:(m + 1) * P],
                                     rwt[:, k, :], start=(k == 0),
                                     stop=(k == CT - 1))
                tmp = wkp.tile([P, E], F32, tag="rtmp")
                nc.vector.tensor_tensor(out=tmp[:], in0=aux[:, 0:E],
                                        in1=se_t[:, 1, :], op=OP.add)
                nc.scalar.activation(tmp[:], tmp[:], AF.Sigmoid)
                nc.vector.tensor_tensor(out=tmp[:], in0=tmp[:], in1=se_t[:, 0, :],
                                        op=OP.mult)
                nc.vector.tensor_tensor(out=bids_sb[:, m, :], in0=tmp[:],
                                        in1=aux[:, E:16], op=OP.add)
            nc.sync.dma_start(out=bids_in.ap().rearrange("(m p) e -> p m e", p=P),
                              in_=bids_sb[:])
            nc.gpsimd.collective_compute(
                "AllGather", OP.bypass, replica_groups=RG,
                ins=[bids_in.ap().opt()], outs=[bids_out.ap().opt()])

            # bridge -> prefix (local tokens); 3-pass at l0, bf16 1-pass at l1
            hp_ = split256(h, "pairA")
            stp = split256(states, "pairB")
            prefix = wkp.tile([P, CT, TOK], F32, tag="sigr", name="prefix")
            npass = 3 if l == 0 else 1
            for m in range(CT):
                w_t = wst.tile([P, 2, 2 * CT, P], BF16, tag="wbpair", bufs=1)
                if l == 0:
                    nc.sync.dma_start(out=w_t[:], in_=bridgeW.ap()[m])
                else:
                    nc.sync.dma_start(out=w_t[:, 0], in_=bridgeW.ap()[m][:, 0])
                ps = w2p.tile([P, CH], F32, space="PSUM", tag=f"w2_{m}")
                i = 0
                for (wh, xh) in ((0, 0), (0, 1), (1, 0))[:npass]:
                    for k in range(2 * CT):
                        rhs = hp_[:, xh, k, :] if k < CT else stp[:, xh, k - CT, :]
                        nc.tensor.matmul(ps[:, 0:TOK], w_t[:, wh, k, :], rhs,
                                         start=(i == 0),
                                         stop=(i == npass * 2 * CT - 1))
                        i += 1
                nc.scalar.activation(prefix[:, m, :], ps[:, 0:TOK], AF.Tanh)

            nc.sync.dma_start(out=pf_in.ap().rearrange("kt p t -> p kt t"),
                              in_=prefix[:])
            nc.gpsimd.collective_compute(
                "AllGather", OP.bypass, replica_groups=RG,
                ins=[pf_in.ap().opt()], outs=[pf_out.ap().opt()])

            # gates from gathered bids: G [E, 16, P], token t = g*128 + p
            bt = wkp.tile([P, 16, E], F32, tag="btile")
            nc.sync.dma_start(out=bt[:],
                              in_=bids_out.ap().rearrange("(g p) e -> p g e", p=P))
            m1 = wkp.tile([P, 2, 16], F32, tag="m1")
            nc.vector.tensor_reduce(m1[:, 0, :], bt[:], axis=AX.X, op=OP.max)
            eq1 = wkp.tile([P, 16, E], F32, tag="eq1")
            nc.vector.tensor_tensor(out=eq1[:], in0=bt[:],
                                    in1=m1[:, 0, :].to_broadcast([P, 16, E]),
                                    op=OP.is_equal)
            msk = wkp.tile([P, 16, E], F32, tag="msk")
            nc.vector.scalar_tensor_tensor(out=msk[:], in0=eq1[:], scalar=-1e30,
                                           in1=bt[:], op0=OP.mult, op1=OP.add)
            nc.vector.tensor_reduce(m1[:, 1, :], msk[:], axis=AX.X, op=OP.max)
            eq2 = wkp.tile([P, 16, E], F32, tag="eq2")
            nc.vector.tensor_tensor(out=eq2[:], in0=msk[:],
                                    in1=m1[:, 1, :].to_broadcast([P, 16, E]),
                                    op=OP.is_equal)
            wg = wkp.tile([P, 2, 16], F32, tag="wg")
            nc.vector.tensor_tensor(out=wg[:, 1, :], in0=m1[:, 1, :],
                                    in1=m1[:, 0, :], op=OP.subtract)
            nc.scalar.activation(wg[:, 1, :], wg[:, 1, :], AF.Sigmoid)
            nc.vector.tensor_scalar(out=wg[:, 0, :], in0=wg[:, 1, :], scalar1=-1.0,
                                    scalar2=1.0, op0=OP.mult, op1=OP.add)
            gt = wkp.tile([P, 16, E], F32, tag="gt")
            nc.vector.tensor_tensor(out=gt[:], in0=eq1[:],
                                    in1=wg[:, 0, :].to_broadcast([P, 16, E]),
                                    op=OP.mult)
            g2t = wkp.tile([P, 16, E], F32, tag="g2t")
            nc.vector.tensor_tensor(out=g2t[:], in0=eq2[:],
                                    in1=wg[:, 1, :].to_broadcast([P, 16, E]),
                                    op=OP.mult)
            nc.vector.tensor_tensor(out=gt[:], in0=gt[:], in1=g2t[:], op=OP.add)
            G = exp.tile([E, 16, P], F32, tag="scr", name="G")
            for g in range(16):
                aux = aux_ps()
                psg = aux[0:E, 0:P]
                nc.tensor.transpose(psg, gt[:, g, :], ident[:])
                nc.vector.tensor_copy(G[:, g, :], psg)
            Gp = wkp.tile([E, 2, 16, P], BF16, tag="Gp")
            nc.vector.tensor_copy(Gp[:, 0], G[:])
            nc.vector.tensor_tensor(out=Gp[:, 1], in0=G[:], in1=Gp[:, 0],
                                    op=OP.subtract)

            # ---- expert phase: NCH chunks of CH tokens, H-sharded dense
            for c in range(NCH):
                # gate plane: [P, E, CH] fp32 broadcast of G rows
                gate_pl = exp.tile([P, E, CH], F32, tag="gatepl")
                gsl = [Gp[:, hh, RPC * 2 * c:RPC * 2 * (c + 1), :].rearrange(
                    "a g p -> a (g p)") for hh in range(2)]
                for e in range(E):
                    gps = w2p.tile([P, CH], F32, space="PSUM", tag=f"w2_{e % CT}")
                    nc.tensor.matmul(gps[:], sel8b[:, e, :], gsl[0],
                                     start=True, stop=False)
                    nc.tensor.matmul(gps[:], sel8b[:, e, :], gsl[1],
                                     start=False, stop=True)
                    nc.vector.tensor_copy(gate_pl[:, e, :], gps[:])

                # gathered h / prefix for this chunk (2 ranks each)
                scr = exp.tile([P, CT, CH], F32, tag="scr")
                hch = exp.tile([P, 2, CT, CH], BF16, tag="hch")
                pfch = exp.tile([P, 2, CT, CH], BF16, tag="pfch")

                def split_ch(dst, src_out):
                    for r in range(RPC):
                        nc.sync.dma_start(
                            out=scr[:, :, r * TOK:(r + 1) * TOK],
                            in_=src_out.ap()[RPC * c + r]
                            .rearrange("kt p t -> p kt t"))
                    nc.vector.tensor_copy(dst[:, 0], scr[:])
                    if l == 0:
                        nc.vector.tensor_tensor(out=dst[:, 1], in0=scr[:],
                                                in1=dst[:, 0], op=OP.subtract)

                split_ch(hch, h_out)
                split_ch(pfch, pf_out)

                npass = 3 if l == 0 else 1
                passes = ((0, 0), (0, 1), (1, 0))[:npass]

                slab_cache = {}

                def load_wslab(ap2, mt, tag):
                    """[MT, P, 2, CT, P] dram -> [P, 2, CT, P] view; paired DMA."""
                    key = (tag, mt)
                    if key in slab_cache:
                        return slab_cache[key]
                    n_mt = ap2.shape[0]
                    lo = mt - (mt % 2)
                    ns = min(2, n_mt - lo)
                    w_t = wst.tile([P, 2, 2, CT, P], BF16, tag=tag, name="wsl",
                                   bufs=(1 if tag in ("apsl", "bsl") else 2))
                    eng = nc.scalar if tag in ("w2sl", "bsl") else nc.sync
                    if l == 0:
                        eng.dma_start(
                            out=w_t[:, 0:ns],
                            in_=ap2[lo:lo + ns].rearrange("s p h kt n -> p s h kt n"))
                    else:
                        eng.dma_start(
                            out=w_t[:, 0:ns, 0],
                            in_=ap2[lo:lo + ns, :, 0].rearrange(
                                "s p kt n -> p s kt n"))
                    for s_ in range(ns):
                        slab_cache[(tag, lo + s_)] = w_t[:, s_]
                    return w_t[:, mt - lo]

                w2ps = [w2p.tile([P, CH], F32, space="PSUM",
                                 tag=f"w2_{m}", name=f"w2ps{m}")
                        for m in range(CT)]
                n_w2 = npass * W2_KT  # mms per output bank

                w2i = [0] * CT

                def w2_accum(slab_pair, kt):
                    w2t = load_wslab(w2s.ap()[l], kt, "w2sl") \
                        if kt < RWKV_MT else \
                        load_wslab(bs_.ap()[l], kt - RWKV_MT, "bsl")
                    for m in range(CT):
                        for (wh, xh) in passes:
                            nc.tensor.matmul(
                                w2ps[m][:], w2t[:, wh, m, :], slab_pair[:, xh, :],
                                start=(w2i[m] == 0), stop=(w2i[m] == n_w2 - 1))
                            w2i[m] += 1

                def post_slab(ps_in, e_lo, e_hi, act):
                    """activation+gate+split -> slab pair [P, 2, CH] bf16."""
                    f1 = stg.tile([P, CH], F32, tag="f1")
                    nc.scalar.activation(f1[:], ps_in, act)
                    if act == AF.Relu:
                        nc.scalar.activation(f1[:], f1[:], AF.Square)
                    slab = slabp.tile([P, 2, CH], BF16, tag="slab")
                    if e_lo == e_hi:
                        nc.vector.tensor_tensor(out=f1[:], in0=f1[:],
                                                in1=gate_pl[:, e_lo, :], op=OP.mult)
                    else:
                        nc.vector.tensor_tensor(out=f1[0:64, :], in0=f1[0:64, :],
                                                in1=gate_pl[0:64, e_lo, :],
                                                op=OP.mult)
                        nc.vector.tensor_tensor(out=f1[64:128, :],
                                                in0=f1[64:128, :],
                                                in1=gate_pl[64:128, e_hi, :],
                                                op=OP.mult)
                    nc.vector.tensor_copy(slab[:, 0], f1[:])
                    if l == 0:
                        nc.vector.tensor_tensor(out=slab[:, 1], in0=f1[:],
                                                in1=slab[:, 0], op=OP.subtract)
                    return slab

                # rwkv ffn: W1 slab -> post -> W2 accumulate
                for mt in range(RWKV_MT):
                    w_t = load_wslab(w1s.ap()[l], mt, "w1sl")
                    ps = psp.tile([P, CH], F32, space="PSUM", tag="mm1", name="ps")
                    i = 0
                    for (wh, xh) in passes:
                        for k in range(CT):
                            nc.tensor.matmul(ps[:], w_t[:, wh, k, :],
                                             hch[:, xh, k, :], start=(i == 0),
                                             stop=(i == npass * CT - 1))
                            i += 1
                    slab = post_slab(ps[:], mt // (HS // P), mt // (HS // P),
                                     AF.Relu)
                    w2_accum(slab, mt)

                # trans experts: pz then a*pz slabs -> B accumulate
                pz = exp.tile([P, TR_MT, CH], F32, tag="pz")
                for mt in range(TR_MT):
                    w_t = load_wslab(aps_.ap()[l], TR_MT + mt, "apsl")
                    ps = psp.tile([P, CH], F32, space="PSUM", tag="mm1", name="ps")
                    i = 0
                    for (wh, xh) in passes:
                        for k in range(CT):
                            nc.tensor.matmul(ps[:], w_t[:, wh, k, :],
                                             pfch[:, xh, k, :], start=(i == 0),
                                             stop=(i == npass * CT - 1))
                            i += 1
                    nc.vector.tensor_copy(pz[:, mt, :], ps[:])
                slab_cache.clear()
                for mt in range(TR_MT):
                    w_t = load_wslab(aps_.ap()[l], mt, "apsl")
                    ps = psp.tile([P, CH], F32, space="PSUM", tag="mm1", name="ps")
                    i = 0
                    for (wh, xh) in passes:
                        for k in range(CT):
                            nc.tensor.matmul(ps[:], w_t[:, wh, k, :],
                                             hch[:, xh, k, :], start=(i == 0),
                                             stop=(i == npass * CT - 1))
                            i += 1
                    f0 = stg.tile([P, CH], F32, tag="f1")
                    nc.scalar.activation(f0[:], ps[:], AF.Silu)
                    nc.vector.tensor_tensor(out=f0[:], in0=f0[:],
                                            in1=pz[:, mt, :], op=OP.mult)
                    # reuse post path minus activation: gate + split
                    e_lo = E_R + (0 if mt == 0 else (0 if mt == 1 else 1))
                    e_hi = E_R + (0 if mt == 0 else 1)
                    slab = slabp.tile([P, 2, CH], BF16, tag="slab")
                    if e_lo == e_hi:
                        nc.vector.tensor_tensor(out=f0[:], in0=f0[:],
                                                in1=gate_pl[:, e_lo, :], op=OP.mult)
                    else:
                        nc.vector.tensor_tensor(out=f0[0:64, :], in0=f0[0:64, :],
                                                in1=gate_pl[0:64, e_lo, :],
                                                op=OP.mult)
                        nc.vector.tensor_tensor(out=f0[64:128, :],
                                                in0=f0[64:128, :],
                                                in1=gate_pl[64:128, e_hi, :],
                                                op=OP.mult)
                    nc.vector.tensor_copy(slab[:, 0], f0[:])
                    if l == 0:
                        nc.vector.tensor_tensor(out=slab[:, 1], in0=f0[:],
                                                in1=slab[:, 0], op=OP.subtract)
                    w2_accum(slab, RWKV_MT + mt)

                # evacuate W2 banks -> rs_in
                for m in range(CT):
                    st = stg.tile([P, CH], F32, tag="f1", name="st")
                    if m % 2 == 0:
                        nc.vector.tensor_copy(st[:], w2ps[m][:])
                    else:
                        nc.scalar.activation(st[:], w2ps[m][:], AF.Copy)
                    for r in range(RPC):
                        nc.sync.dma_start(
                            out=rs_in.ap()[RPC * c + r, m],
                            in_=st[:, r * TOK:(r + 1) * TOK])
            nc.gpsimd.collective_compute(
                "ReduceScatter", OP.add, replica_groups=RG,
                ins=[rs_in.ap().opt()], outs=[rs_out.ap().opt()])
            moe = wkp.tile([P, CT, TOK], F32, tag="vv", name="moe")
            nc.sync.dma_start(out=moe[:],
                              in_=rs_out.ap().rearrange("kt p t -> p kt t"))
            nc.vector.tensor_tensor(out=x[:], in0=x[:], in1=moe[:], op=OP.add)

        # final layernorm + allgather + head
        xf = layer_norm(x, lnos.ap(), lnob.ap())
        xfc = wkp.tile([P, CT, TOK], F32, tag="vv", name="xfc")
        nc.vector.tensor_copy(xfc[:], xf[:])
        nc.sync.dma_start(out=xf_in.ap().rearrange("kt p t -> p kt t"), in_=xfc[:])
        nc.gpsimd.collective_compute(
            "AllGather", OP.bypass, replica_groups=RG,
            ins=[xf_in.ap().opt()], outs=[xf_out.ap().opt()])
        for c4 in range(NCH):
            xfch = exp.tile([P, CT, CH], BF16, tag="xfch", bufs=1)
            for r in range(RPC):
                nc.gpsimd.dma_start(
                    out=xfch[:, :, r * TOK:(r + 1) * TOK],
                    in_=xf_out.ap()[RPC * c4 + r].rearrange("kt p t -> p kt t"))
            for m in range(HEAD_MT):
                w_t = wst.tile([P, CT, P], BF16, tag="whead")
                nc.sync.dma_start(out=w_t[:], in_=headw.ap()[m])
                ps = w2p.tile([P, CH], F32, space="PSUM", tag=f"w2_{m % CT}")
                for k in range(CT):
                    nc.tensor.matmul(ps[:], w_t[:, k, :], xfch[:, k, :],
                                     start=(k == 0), stop=(k == CT - 1))
                st = stg.tile([P, CH], BF16, tag="f1", name="st")
                if m % 2 == 0:
                    nc.vector.tensor_copy(st[:], ps[:])
                else:
                    nc.scalar.activation(st[:], ps[:], AF.Copy)
                nc.sync.dma_start(
                    out=logitsT.ap()[m * P:(m + 1) * P, c4 * CH:(c4 + 1) * CH],
                    in_=st[:])
    finally:
        for p_ in reversed(ctxs):
            p_.__exit__(None, None, None)
        tc.__exit__(None, None, None)

    nc.compile()
    return nc


def _sel8_const():
    s = np.zeros((E, E, P), np.float32)
    for e in range(E):
        s[e, e, :] = 1.0
    return s


def _pair(a):
    """f32 array -> [2, ...] bf16 hi/lo."""
    bf = ml_dtypes.bfloat16
    a = np.asarray(a, np.float32)
    hi = a.astype(bf)
    lo = (a - hi.astype(np.float32)).astype(bf)
    return np.stack([hi, lo], axis=0)


def _mlay(W):
    """[..., R, Nc] -> [..., Nc//P, P, R//P, P]  (m, p, kt, n)."""
    W = np.asarray(W)
    R, Nc = W.shape[-2:]
    lead = W.shape[:-2]
    W = W.reshape(*lead, R // P, P, Nc // P, P)
    nl = len(lead)
    perm = tuple(range(nl)) + (nl + 2, nl + 1, nl + 0, nl + 3)
    return np.ascontiguousarray(W.transpose(*perm))


def _host_prep(inputs):
    f32 = np.float32
    idx = np.asarray(inputs["idx"]).astype(np.int64)
    emb_W = np.asarray(inputs["emb_W"], dtype=f32)
    x0 = emb_W[idx.reshape(-1)]                      # [M, C]
    decay = np.asarray(inputs["decay"], dtype=f32)
    d = (1.0 / (1.0 + np.exp(-decay.astype(np.float64)))).astype(np.float64)
    caps = np.asarray(inputs["capital_shares"], dtype=f32)
    shares = caps / caps.sum(axis=1, keepdims=True)  # [L, E]

    def chanlay(a):
        a = np.asarray(a, dtype=f32)
        return np.ascontiguousarray(a.reshape(*a.shape[:-1], CT, P).swapaxes(-1, -2))

    conf_w = np.asarray(inputs["conf_w"], dtype=f32)
    critic = np.asarray(inputs["critic_Wa"], dtype=f32)
    routerW = np.ascontiguousarray(
        np.concatenate([conf_w.transpose(0, 2, 1), critic], axis=2))

    wmix = np.stack([np.asarray(inputs[k], f32)
                     for k in ("Wr", "Wk", "Wv", "Wo")], axis=1)  # [L,4,C,C]
    wmix = _mlay(_pair(wmix)).transpose(1, 2, 3, 4, 0, 5, 6)      # [L,4,m,P,2,kt,P]

    ffn_W1 = np.asarray(inputs["ffn_W1"], dtype=f32)
    ffn_W2 = np.asarray(inputs["ffn_W2"], dtype=f32)
    tA = np.asarray(inputs["trans_A"], dtype=f32)
    tP = np.asarray(inputs["trans_P"], dtype=f32)
    tB = np.asarray(inputs["trans_B"], dtype=f32)
    head_W = np.asarray(inputs["head_W"], dtype=f32)
    conf_b = np.asarray(inputs["conf_b"], dtype=f32)  # [L, E]

    shared = dict(
        wmix=np.ascontiguousarray(wmix),
        dsig=chanlay(d.astype(f32)),
        ln1s=chanlay(inputs["ln1_s"]), ln1b=chanlay(inputs["ln1_b"]),
        ln2s=chanlay(inputs["ln2_s"]), ln2b=chanlay(inputs["ln2_b"]),
        lnos=chanlay(inputs["lnout_s"]), lnob=chanlay(inputs["lnout_b"]),
        routerW=routerW,
        confb=np.ascontiguousarray(
            np.broadcast_to(conf_b[:, None, :], (L, P, E)).astype(f32)),
        sharesb=np.ascontiguousarray(
            np.broadcast_to(shares[:, None, :], (L, P, E)).astype(f32)),
        bridgeW=np.ascontiguousarray(
            _mlay(_pair(inputs["bridge_W"])).transpose(1, 2, 0, 3, 4)),
        sel8c=_sel8_const(),
        onescol=np.ones((P, 1), np.float32),
        onesrow=np.ones((1, P), np.float32),
    )

    in_maps = []
    for i in range(NC_):
        b_idx, j = divmod(i, NC_ // B)
        scanP_l = np.zeros((L, C, NC_), np.float64)
        for ll in range(L):
            for mprev in range(j):
                ridx = b_idx * (NC_ // B) + mprev
                scanP_l[ll, :, ridx] = d[ll] ** (256.0 * (j - mprev - 1))
        scanP_lay = np.ascontiguousarray(
            scanP_l.astype(f32).reshape(L, CT, P, NC_).swapaxes(1, 2))

        w1c = ffn_W1[:, :, :, i * HS:(i + 1) * HS]
        w1c = np.ascontiguousarray(w1c.transpose(0, 2, 1, 3).reshape(L, C, E_R * HS))
        w2c = np.ascontiguousarray(
            ffn_W2[:, :, i * HS:(i + 1) * HS, :].reshape(L, E_R * HS, C))
        a_s = tA[:, :, :, i * H2S:(i + 1) * H2S].transpose(0, 2, 1, 3)
        a_s = a_s.reshape(L, C, E_T * H2S)
        p_s = tP[:, :, :, i * H2S:(i + 1) * H2S].transpose(0, 2, 1, 3)
        p_s = p_s.reshape(L, C, E_T * H2S)
        apsc = np.ascontiguousarray(np.concatenate([a_s, p_s], axis=2))
        b_c = np.ascontiguousarray(
            tB[:, :, i * H2S:(i + 1) * H2S, :].reshape(L, E_T * H2S, C))

        hw = np.zeros((C, VSP), f32)
        lo = i * VS
        hi = min((i + 1) * VS, V)
        hw[:, :hi - lo] = head_W[:, lo:hi]

        x0T = np.ascontiguousarray(
            x0[i * TOK:(i + 1) * TOK].T.reshape(CT, P, TOK))

        im = dict(shared)
        im.update(
            x0T=x0T.astype(f32),
            scanP=scanP_lay,
            w1s=np.ascontiguousarray(
                _mlay(_pair(w1c)).transpose(1, 2, 3, 0, 4, 5)),
            w2s=np.ascontiguousarray(
                _pair(w2c).reshape(2, L, RWKV_MT, P, CT, P)
                .transpose(1, 2, 3, 0, 4, 5)),
            aps=np.ascontiguousarray(
                _mlay(_pair(apsc)).transpose(1, 2, 3, 0, 4, 5)),
            bs=np.ascontiguousarray(
                _pair(b_c).reshape(2, L, TR_MT, P, CT, P)
                .transpose(1, 2, 3, 0, 4, 5)),
            headw=np.ascontiguousarray(_mlay(hw).astype(ml_dtypes.bfloat16)),
        )
        in_maps.append(im)
    return in_maps


def kernel(**inputs):
    if "nc" not in _CACHE:
        _CACHE["nc"] = _build()
    nc = _CACHE["nc"]
    in_maps = _host_prep(inputs)
    trace = os.environ.get("K_TRACE", "0") == "1"
    res = bass_utils.run_bass_kernel_spmd(nc, in_maps, core_ids=list(range(NC_)),
                                          trace=trace)
    _CACHE["last_res"] = res
    outs = []
    for i in range(NC_):
        lt = np.asarray(res.results[i]["logitsT"], dtype=np.float32)
        lo = i * VS
        hi = min((i + 1) * VS, V)
        outs.append(lt[: hi - lo].T)
    full = np.concatenate(outs, axis=1)
    return full.reshape(B, T, V).astype(np.float32)
